# revision 72
# baseline (speedup 1.0000x reference)
"""Trainium2 Bass kernel for nn_Block_19301583028789.

Pipeline (per batch element): channel-mixing Linear -> erf-GELU -> S4D (FFT conv
in the reference; computed here as an exact chunked linear recurrence) -> FiLM
-> erf-GELU -> per-channel residual.

Sharding: data-parallel over batch B=16 across 8 cores (2 batches/core);
all parameters replicated.

S4D math: split L into C=128 chunks of T=128.  Per chunk: local causal conv =
lower-tri Toeplitz matmul; cross-chunk carry = rank-2N apply of the complex
mode states S[n,c] = sum_{c'<=c} mu^{c-c'} Z[n,c'], Z = per-chunk Vandermonde
summary (matmul), mu = lam^T.

The chunk-state recurrence S_c = mu*S_{c-1} + Z_c (complex mu) is decoupled
into two REAL recurrences via a modulus/phase split: with mu = rho*e^{i*theta},
pre-twist Zt_c = e^{-ic*theta} Z_c, then St_c = rho*St_{c-1} + Zt_c (real rho,
re/im independent -> DVE tensor_tensor_scan hardware prefix scan), then
post-twist S_c = e^{+ic*theta} St_c, whose adds fold into the carry matmuls.

Layouts: scan state Zt is [128=(b,n) partitions, (p=re/im, h, c) free] so all
complex cross-terms are same-partition free-offset reads (the BIR verifier
forbids cross-partition tensor ops).  All matmuls run in bf16 (1 cyc/row).

The FiLM scale g (computable on host from conditional_information) is folded
into the Toeplitz/carry lhsT tiles per batch; the FiLM bias is applied as a
rank-1 ones-outer-product matmul accumulated into the conv PSUM, so the final
GELU is a single unparameterized activation per (batch, 4-channel group).
The per-channel residual res_w*x is accumulated into the output transpose's
PSUM bank via an identity matmul.  y is stored bf16 in a DMA-friendly
[b, oct, c, (hh, t)] layout and reassembled on host.
"""

import numpy as np

import concourse.bass as bass
import concourse.tile as tile
import concourse.mybir as mybir
from concourse.bass_utils import run_bass_kernel_spmd

B, H, L = 16, 64, 16384
N, CD = 64, 32
T = 128
C = L // T           # 128 chunks
NCORES = 8
BLOC = B // NCORES   # 2
NOCT = 8             # h-octets
HOCT = H // NOCT     # 8 channels per octet
HC = HOCT * C        # 1024: octet's (h, c) free span
FP32 = mybir.dt.float32
BF16 = mybir.dt.bfloat16
AF = mybir.ActivationFunctionType
ALU = mybir.AluOpType

_CACHE = {}


def _split_multi_waits(nc, max_waits=1):
    """Walrus TPB lowering accepts only 1 sync-wait per instruction for most
    opcodes; Tile can accumulate one wait per producer engine.  Hoist extras
    onto NoOps inserted right before the offending instruction."""
    for fn in nc.m.functions:
        for blk in fn.blocks:
            insts = blk.instructions
            i = 0
            while i < len(insts):
                inst = insts[i]
                si = inst.sync_info
                if si is not None and len(si.on_wait) > max_waits:
                    extra = list(si.on_wait[:-max_waits])
                    keep = list(si.on_wait[-max_waits:])
                    nops = [
                        mybir.InstNoOp(
                            name=f"{inst.name}-waitsplit{k}",
                            opcode="NoOp",
                            engine=inst.engine,
                            sync_info=mybir.SyncInfo(on_wait=[w], on_update=[]),
                        )
                        for k, w in enumerate(extra)
                    ]
                    si.on_wait = keep
                    for k, nop in enumerate(nops):
                        insts.insert(i + k, nop)
                    i += len(nops)
                i += 1


def _host_params(log_dt, log_A_real, A_imag, C_re, C_im, D, W_lin, b_lin):
    """Parameter-derived constant tensors (fp64 host math), in SBUF layouts."""
    import ml_dtypes
    bf = lambda a: np.ascontiguousarray(a.astype(np.float32), dtype=ml_dtypes.bfloat16)
    f32 = lambda a: np.ascontiguousarray(a, dtype=np.float32)

    dt = np.exp(log_dt.astype(np.float64))[:, None]            # [H,1]
    A = -np.exp(log_A_real.astype(np.float64)) + 1j * A_imag.astype(np.float64)
    dtA = A * dt                                               # [H,N]
    coef = (C_re.astype(np.float64) + 1j * C_im.astype(np.float64)) \
        * (np.exp(dtA) - 1.0) / A                              # [H,N]

    ks = np.arange(T + 2)
    lp = np.exp(dtA[:, :, None] * ks[None, None, :])           # [H,N,T+2]

    # K kernel first T taps; fold D into tap 0
    K = 2.0 * np.real(np.einsum("hn,hnm->hm", coef, lp[:, :, :T]))  # [H,T]
    K[:, 0] += D.astype(np.float64)

    # Toeplitz lhsT[j,t] = K[t-j] (t>=j); FiLM scale g applied per core
    idx = np.arange(T)
    tm = idx[None, :] - idx[:, None]                           # [j,t]
    Ktoep = np.where(tm >= 0, K[:, np.clip(tm, 0, T - 1)], 0.0)  # [H,j,t]

    # Z summary lhsT: lam^(T-1-t); layout [oct, t, (hh, p, n)]
    Alq = lp[:, :, ::-1][:, :, 2:T + 2]                        # lam^(T-1-t): [H,N,T]
    Aq = np.stack([np.transpose(Alq.real, (0, 2, 1)),
                   np.transpose(Alq.imag, (0, 2, 1))], axis=2)  # [H,T,2,N]
    Aqq = np.transpose(Aq.reshape(NOCT, HOCT, T, 2 * N), (0, 2, 1, 3)) \
        .reshape(NOCT, T, HOCT * 2 * N)

    # carry apply: Re / -Im of 2*coef*lam^(t+1); FiLM-scaled per core
    P = 2.0 * coef[:, :, None] * lp[:, :, 1:T + 1]             # [H,N,T]

    # chunk transition mu = lam^T = rho*e^{i theta}; [2N(b-dup), (h, c)]
    rho = np.exp(T * dtA.real)                                 # [H,N]
    theta = T * dtA.imag
    cs = np.arange(C)
    ang = theta[:, :, None] * cs[None, None, :]                # [H,N,C]
    dup = lambda a: np.concatenate([a, a], axis=0).reshape(2 * N, H * C)
    cosq = dup(np.transpose(np.cos(ang), (1, 0, 2)))
    sinN = dup(np.transpose(-np.sin(ang), (1, 0, 2)))
    r0 = np.broadcast_to(rho.T[:, :, None], (N, H, C)).copy()
    r0[:, :, 0] = 0.0                                          # segment resets
    rho0 = dup(r0)

    return {
        "Ktoep": Ktoep, "P_re": P.real, "P_imn": -P.imag,
        "Aqq": bf(Aqq), "cosq": bf(cosq), "sinN": bf(sinN), "rho0": bf(rho0),
        "WBq": bf(np.concatenate([W_lin.T, b_lin[None, :]], 0)),   # [H+1,H]
    }


def _build():
    nc = bass.Bass("TRN2", target_bir_lowering=False, debug=False)

    def din(name, shape, dtype=FP32):
        return nc.dram_tensor(name, list(shape), dtype, kind="ExternalInput")

    x_in = din("x_loc", [BLOC, H + 1, L], BF16)        # ones channel appended
    x_res = din("x_res", [BLOC, NOCT, C, HOCT * T], BF16)  # res_w * x
    WB = din("WBq", [H + 1, H], BF16)
    Ktq = din("Ktq", [BLOC, NOCT, T, HOCT * T], BF16)
    Aqq = din("Aqq", [NOCT, T, HOCT * 2 * N], BF16)
    Prq = din("Prq", [NOCT, 2 * N, HOCT * T], BF16)
    Piq = din("Piq", [NOCT, 2 * N, HOCT * T], BF16)
    rho0 = din("rho0", [2 * N, H * C], BF16)
    cosq = din("cosq", [2 * N, H * C], BF16)
    sinN = din("sinN", [2 * N, H * C], BF16)
    eye = din("eye", [128, 128])
    eyeb = din("eyeb", [128, 128], BF16)
    biasq = din("biasq", [BLOC, H * C], BF16)          # FiLM bias, c-repeated
    # y in DMA-friendly [b, oct, c, (hh, t)] bf16 layout; host reassembles
    y_out = nc.dram_tensor("y_out", [BLOC, NOCT, C, HOCT * T], BF16,
                           kind="ExternalOutput")

    xv = x_in.ap().rearrange("b h (q l) -> b h q l", q=8)      # 8 col-eighths
    xrv = x_res.ap()
    yv = y_out.ap()

    with tile.TileContext(nc) as tc:
        with (
            tc.tile_pool(name="big", bufs=1) as big,
            tc.tile_pool(name="xhl", bufs=2) as xhl,
            tc.tile_pool(name="par", bufs=2) as par,
            tc.tile_pool(name="tmp", bufs=2) as tmp,
            tc.tile_pool(name="qt", bufs=2) as qt,
            tc.tile_pool(name="ev", bufs=2) as ev,
            tc.tile_pool(name="xr", bufs=2) as xr,
            tc.tile_pool(name="cst", bufs=1) as cst,
            tc.tile_pool(name="ps_w", bufs=2, space="PSUM") as ps_w,
            tc.tile_pool(name="ps_tr", bufs=2, space="PSUM") as ps_tr,
            tc.tile_pool(name="ps_z1", bufs=2, space="PSUM") as ps_z1,
        ):
            # ---- resident tensors ----
            u = big.tile([128, BLOC * H * C], BF16, tag="u")       # [t,(b,h,c)]
            uv = u[:].rearrange("t (b h c) -> t b h c", b=BLOC, h=H)
            Zt = big.tile([2 * N, 2 * H * C], BF16, tag="Zt")      # [(b,n),(p,h,c)]
            Ztv = Zt[:].rearrange("q (p h c) -> q p h c", p=2, h=H)
            rho_sb = big.tile([2 * N, H * C], BF16, tag="rho")
            cos_sb = big.tile([2 * N, H * C], BF16, tag="cos")
            sin_sb = big.tile([2 * N, H * C], BF16, tag="sin")
            cos_v = cos_sb[:].rearrange("q (h c) -> q h c", h=H)
            sin_v = sin_sb[:].rearrange("q (h c) -> q h c", h=H)

            wb_sb = cst.tile([H + 1, H], BF16, tag="wb")
            nc.sync.dma_start(wb_sb[:], WB.ap())

            def load_par(o):
                kt = par.tile([T, BLOC * HOCT * T], BF16, tag="kt")
                nc.sync.dma_start(kt[:].rearrange("t (b f) -> t b f", b=BLOC),
                                  Ktq.ap()[:, o].rearrange("b t f -> t b f"))
                aq = par.tile([T, HOCT * 2 * N], BF16, tag="aq")
                nc.sync.dma_start(aq[:], Aqq.ap()[o])
                pre = par.tile([2 * N, HOCT * T], BF16, tag="pre")
                nc.sync.dma_start(pre[:], Prq.ap()[o])
                pim = par.tile([2 * N, HOCT * T], BF16, tag="pim")
                nc.sync.dma_start(pim[:], Piq.ap()[o])
                bias_sb = par.tile([1, BLOC * HOCT * C], BF16, tag="bias")
                nc.sync.dma_start(
                    bias_sb[:].rearrange("p (b f) -> p b f", b=BLOC),
                    biasq.ap().rearrange("b (o f) -> o b f", o=NOCT)[o][None, :, :])
                osl = slice(o * HC, (o + 1) * HC)
                nc.sync.dma_start(cos_sb[:, osl], cosq.ap()[:, osl])
                nc.sync.dma_start(sin_sb[:, osl], sinN.ap()[:, osl])
                nc.sync.dma_start(rho_sb[:, osl], rho0.ap()[:, osl])
                return kt, aq, pre, pim, bias_sb

            # ---- phase A: u = gelu(W x + b), transposed to [t,(b,h,c)] ----
            CQ = C // 8   # chunks per x-eighth (16)
            par0 = par1 = None
            for b in range(BLOC):
                if b == 1:
                    par0 = load_par(0)
                for q in range(8):
                    xt = xhl.tile([H + 1, CQ * T], BF16, tag="xt")
                    nc.sync.dma_start(xt[:], xv[b, :, q, :])
                    for c8 in range(CQ // 8):          # PSUM groups of 8 chunks
                        wp = ps_w.tile([T, 8 * H], FP32, tag="wp")
                        for k in range(8):
                            cc = c8 * 8 + k
                            nc.tensor.matmul(
                                wp[:, k * H:(k + 1) * H],
                                xt[:, cc * T:(cc + 1) * T], wb_sb[:],
                                start=True, stop=True, skip_group_check=True)
                        c0 = q * CQ + c8 * 8
                        nc.scalar.activation(
                            uv[:, b, :, c0:c0 + 8],
                            wp[:].rearrange("t (c h) -> t h c", c=8),
                            AF.Gelu)

            # ---- transpose identities + FiLM bias table (queued behind x) ----
            eye_sb = cst.tile([128, 128], FP32, tag="eye")
            eyeb_sb = cst.tile([128, 128], BF16, tag="eyeb")
            nc.sync.dma_start(eye_sb[:], eye.ap())
            nc.sync.dma_start(eyeb_sb[:], eyeb.ap())
            ones_sb = cst.tile([1, T], BF16, tag="onesT")
            nc.vector.memset(ones_sb[:], 1.0)



            # ---- per h-octet: Z, twist, scan, untwist+carry, conv, out ----
            for o in range(NOCT):
                h0 = o * HOCT
                kt, aq, pre, pim, bias_sb = par0 if o == 0 else load_par(o)

                # -- Z summaries: out [(b,n) part, c] per (h, p); ACT/DVE copy --
                for p in range(2):
                    for quad in range(2):
                        zp = ps_w.tile([2 * N, 4 * C], FP32, tag="wp")
                        for k in range(4):
                            hh = quad * 4 + k
                            lhs = aq[:, (hh * 2 + p) * N:(hh * 2 + p + 1) * N]
                            for b in range(BLOC):
                                nc.tensor.matmul(
                                    zp[b * N:(b + 1) * N, k * C:(k + 1) * C],
                                    lhs, uv[:, b, h0 + hh, :],
                                    start=True, stop=True, skip_group_check=True)
                        dst = Ztv[:, p, h0 + quad * 4:h0 + quad * 4 + 4, :] \
                            .rearrange("q h c -> q (h c)")
                        if quad == 0:
                            nc.scalar.copy(dst, zp[:])
                        else:
                            nc.vector.tensor_copy(dst, zp[:])

                zsl0 = Zt[:, o * HC:(o + 1) * HC]                  # re block
                zsl1 = Zt[:, H * C + o * HC:H * C + (o + 1) * HC]  # im block
                co = cos_v[:, h0:h0 + HOCT, :]
                si = sin_v[:, h0:h0 + HOCT, :]
                cob = co[:, None, :, :].broadcast_to([2 * N, 2, HOCT, C])
                zall = Ztv[:, :, h0:h0 + HOCT, :]

                # -- pre-twist: Z <- e^{-ic theta} Z  (sinN = -sin) --
                #   re' = Zre*cos - Zim*sinN ; im' = Zim*cos + Zre*sinN
                t1 = tmp.tile([2 * N, 2 * HC], BF16, tag="t1")
                t2 = tmp.tile([2 * N, 2 * HC], BF16, tag="t2")
                t1v = t1[:].rearrange("q (p h c) -> q p h c", p=2, h=HOCT)
                nc.vector.tensor_mul(t1v, zall, cob)
                t2v = t2[:].rearrange("q (p h c) -> q p h c", p=2, h=HOCT)
                nc.vector.tensor_mul(t2v[:, 0, :, :], zall[:, 1, :, :], si)
                nc.gpsimd.tensor_mul(t2v[:, 1, :, :], zall[:, 0, :, :], si)
                nc.vector.tensor_sub(zsl0, t1[:, 0:HC], t2[:, 0:HC])
                nc.vector.tensor_add(zsl1, t1[:, HC:2 * HC], t2[:, HC:2 * HC])

                # -- real-decay prefix scan along chunks (resets at c=0) --
                rsl = rho_sb[:, o * HC:(o + 1) * HC]
                nc.vector.tensor_tensor_scan(zsl0, rsl, zsl0, 0.0, ALU.mult, ALU.add)
                nc.vector.tensor_tensor_scan(zsl1, rsl, zsl1, 0.0, ALU.mult, ALU.add)

                # -- post-twist products; the +/- recombination folds into the
                #    carry matmuls:  Sre = q1 + q2,  Sim = q3 - q4 --
                q1 = qt.tile([2 * N, HC], BF16, tag="q1")   # St_re * cos
                q2 = qt.tile([2 * N, HC], BF16, tag="q2")   # St_im * sinN
                q4t = tmp.tile([2 * N, 2 * HC], BF16, tag="t2")
                q3 = q4t[:, HC:2 * HC]                      # St_im * cos
                q4 = q4t[:, 0:HC]                           # St_re * sinN
                q1v = q1[:].rearrange("q (h c) -> q h c", h=HOCT)
                q2v = q2[:].rearrange("q (h c) -> q h c", h=HOCT)
                q3v = q3.rearrange("q (h c) -> q h c", h=HOCT)
                q4v = q4.rearrange("q (h c) -> q h c", h=HOCT)
                zv0 = zall[:, 0, :, :]
                zv1 = zall[:, 1, :, :]
                nc.vector.tensor_mul(q1v, zv0, co)
                nc.gpsimd.tensor_mul(q2v, zv1, si)
                nc.vector.tensor_mul(q3v, zv1, co)
                nc.vector.tensor_mul(q4v, zv0, si)
                sim = qt.tile([2 * N, HC], BF16, tag="sim")
                nc.vector.tensor_sub(sim[:], q3, q4)

                # -- conv + carry (FiLM-scaled params) + bias + GELU
                #    + transpose + residual + store --
                xcp0 = xr.tile([128, HOCT * T], BF16, tag="xc")
                nc.sync.dma_start(xcp0[:], xrv[0, o])
                xcp1 = xr.tile([128, HOCT * T], BF16, tag="xc")
                nc.sync.dma_start(xcp1[:], xrv[1, o])
                for b in range(BLOC):
                    tp = ps_tr.tile([128, HOCT * T], FP32, tag="tp")
                    xc = xcp0 if b == 0 else xcp1
                    bn = slice(b * N, (b + 1) * N)
                    for quad in range(2):
                        z1 = ps_z1.tile([T, 4 * C], FP32, tag="z1")
                        bc0 = (b * HOCT + quad * 4) * C
                        nc.tensor.matmul(z1[:], ones_sb[:],
                                         bias_sb[:, bc0:bc0 + 4 * C],
                                         start=True, stop=False,
                                         skip_group_check=True)
                        for k in range(4):
                            hh = quad * 4 + k
                            zs = slice(k * C, (k + 1) * C)
                            zw = slice(k * C + 1, (k + 1) * C)
                            pslc = slice((b * HOCT + hh) * T,
                                         (b * HOCT + hh + 1) * T)
                            cslc = slice(hh * T, (hh + 1) * T)
                            wnd = slice(hh * C, hh * C + C - 1)
                            nc.tensor.matmul(
                                z1[:, zs], kt[:, pslc],
                                uv[:, b, h0 + hh, :], start=False, stop=False,
                                skip_group_check=True)
                            nc.tensor.matmul(z1[:, zw], pre[bn, cslc], q1[bn, wnd],
                                             start=False, stop=False,
                                             skip_group_check=True)
                            nc.tensor.matmul(z1[:, zw], pre[bn, cslc], q2[bn, wnd],
                                             start=False, stop=False,
                                             skip_group_check=True)
                            nc.tensor.matmul(z1[:, zw], pim[bn, cslc], sim[bn, wnd],
                                             start=False, stop=(k == 3),
                                             skip_group_check=True)
                        yg = ev.tile([T, 4 * C], FP32, tag="yg")
                        nc.scalar.activation(yg[:], z1[:], AF.Gelu)
                        for k in range(4):
                            hh = quad * 4 + k
                            cslc = slice(hh * T, (hh + 1) * T)
                            nc.tensor.matmul(tp[:, cslc], yg[:, k * C:(k + 1) * C],
                                             eye_sb[:], is_transpose=True,
                                             start=True, stop=False,
                                             skip_group_check=True)
                            nc.tensor.matmul(tp[:, cslc], eyeb_sb[:], xc[:, cslc],
                                             start=False, stop=True,
                                             skip_group_check=True)
                    yo = xr.tile([128, HOCT * T], BF16, tag="yo")
                    if b == 0:
                        nc.scalar.copy(yo[:], tp[:])
                    else:
                        nc.vector.tensor_copy(yo[:], tp[:])
                    nc.scalar.dma_start(yv[b, o], yo[:])

    _split_multi_waits(nc)
    return nc


def kernel(**inputs):
    import ml_dtypes
    key = "k"
    if key not in _CACHE:
        _CACHE[key] = _build()
    nc = _CACHE[key]

    hp = _host_params(
        inputs["log_dt"], inputs["log_A_real"], inputs["A_imag"],
        inputs["C_re"], inputs["C_im"], inputs["D"],
        inputs["W_lin"], inputs["b_lin"])

    x = np.ascontiguousarray(inputs["x"], dtype=np.float32)
    cond = np.ascontiguousarray(inputs["conditional_information"], dtype=np.float32)
    film_W = np.ascontiguousarray(inputs["film_W"], dtype=np.float32)
    film_b = np.ascontiguousarray(inputs["film_b"], dtype=np.float32)
    res_w = np.ascontiguousarray(inputs["res_w"], dtype=np.float32)

    bf = lambda a: np.ascontiguousarray(np.asarray(a, dtype=np.float32)
                                        .astype(ml_dtypes.bfloat16))

    # FiLM params on host: gb = cond @ film_W.T + film_b -> scale g, bias
    gb = cond @ film_W.T + film_b[None, :]                     # [B, 2H]
    g_all, bias_all = gb[:, :H], gb[:, H:]

    # x with ones channel (for the Linear bias row in the [H+1,H] weight)
    x_aug = bf(np.concatenate([x, np.ones((B, 1, L), np.float32)], axis=1))
    # res_w * x in [b, oct, c, (hh, t)] layout for the post-transpose residual
    x_rs = (x * res_w[None, :, None]).reshape(B, NOCT, HOCT, C, T)
    x_rs = bf(np.transpose(x_rs, (0, 1, 3, 2, 4)).reshape(B, NOCT, C, HOCT * T))

    common = {
        "Aqq": hp["Aqq"], "cosq": hp["cosq"], "sinN": hp["sinN"],
        "rho0": hp["rho0"], "WBq": hp["WBq"],
        "eye": np.eye(128, dtype=np.float32),
        "eyeb": bf(np.eye(128, dtype=np.float32)),
    }
    Ktoep, P_re, P_imn = hp["Ktoep"], hp["P_re"], hp["P_imn"]

    def pq(v):  # [BLOC,H,N,T] (g-scaled) -> [NOCT, 2N, HOCT*T]
        q = np.transpose(v.reshape(BLOC, NOCT, HOCT, N, T), (0, 1, 3, 2, 4)) \
            .reshape(BLOC, NOCT, N, HOCT * T)
        return np.concatenate([q[0], q[1]], axis=1)            # [NOCT,2N,HOCT*T]

    in_maps = []
    for c_ in range(NCORES):
        g = g_all[c_ * BLOC:(c_ + 1) * BLOC]                   # [BLOC, H]
        bias = bias_all[c_ * BLOC:(c_ + 1) * BLOC]
        ktg = Ktoep[None, :, :, :] * g[:, :, None, None]       # [BLOC,H,T,T]
        ktg = np.transpose(ktg.reshape(BLOC, NOCT, HOCT, T, T),
                           (0, 1, 3, 2, 4)).reshape(BLOC, NOCT, T, HOCT * T)
        m = dict(common)
        m["Ktq"] = bf(ktg)
        m["Prq"] = bf(pq(P_re[None] * g[:, :, None, None]))
        m["Piq"] = bf(pq(P_imn[None] * g[:, :, None, None]))
        m["biasq"] = bf(np.repeat(bias[:, :, None], C, axis=2).reshape(BLOC, H * C))
        m["x_loc"] = np.ascontiguousarray(x_aug[c_ * BLOC:(c_ + 1) * BLOC])
        m["x_res"] = np.ascontiguousarray(x_rs[c_ * BLOC:(c_ + 1) * BLOC])
        in_maps.append(m)

    res = run_bass_kernel_spmd(nc, in_maps, core_ids=list(range(NCORES)))
    out = np.concatenate([res.results[c_]["y_out"] for c_ in range(NCORES)], axis=0)
    # [B, oct, c, hh, t] -> [B, (oct, hh), (c, t)]
    out = out.astype(np.float32).reshape(B, NOCT, C, HOCT, T)
    out = np.transpose(out, (0, 1, 3, 2, 4)).reshape(B, H, L)
    return np.ascontiguousarray(out)
